# revision 79
# baseline (speedup 1.0000x reference)
"""Trainium2 Bass kernel for nn_BuddingLayer (moe_routing).

Computation (B=512, SIN=SOUT=2048, K=128 buds):
  dense = (x * ~mask) @ weight.T + bias          mask = one-hot(sat_idx)
  per bud k (v = x[:, sat_idx[k]]):
    h1 = relu(v * c1[k] + b1[k])                 c1[k,j] = sum_i W1[k,i,j]/3
    h2 = relu(h1 @ W2[k] + b2[k])                [B, 3]
    u += relu(h2 @ W3[k] + b3[k])                [B, 2048]
  out = dense + u

Sharding: output-feature split, 256 columns per core (8 cores); compute in
transposed layout [o_part, b_free].  Host does slicing/permutation only; all
math runs on device.

Bud supertile t packs buds {t, 32+t, 64+t, 96+t} at PE row-group bases
{0,32,64,96}; each bud's 4-row lhsT carries 3 W3 rows + b3 (against a
constant-1 rhs row), so bias rides the matmul for free.  w3b is host-permuted
and loaded by one cast-DMA; h2t needs only 4 pack DMAs (j-row gather).

z flows through a 4-deep [128,1024] PSUM tag-ring (2 matmuls per tile).  The
dense path runs as two paced bursts through the same ring, copied to SBUF by
ScalarE Identity(+bias) in its startup lull and merged once on GpSimd.  Per-exit Bresenham
split of the 128 z-exits:
  'A' exits: ScalarE relu -> fp16 ring slots; slot pairs are summed into
     acc16 lanes by SWDGE cce-add DMAs (sum runs on the DMA engines, off the
     compute engines);
  'D' exits: VectorE fused relu+accumulate (scalar_tensor_tensor max/add)
     into fp32 lanes;
  'T' exits (trailing units, grouped oc0-then-oc1): ScalarE relu + immediate
     VectorE tensor_tensor fold, so no DMA chain dangles at the end.
Lane folds + final dense+bud combine run on VectorE per oc as soon as that
oc's units finish; per-oc output stores overlap the other oc's tail.
"""

import numpy as np

N_CORES = 8
B = 512
SIN = 2048
SOUT = 2048
K = 128
OC = SOUT // N_CORES          # 256 output cols per core
NCHUNK = SIN // 128           # 16 contraction chunks for dense
NT = K // 4                   # 32 supertiles (4 buds each)
NU = 2 * NT                   # 64 z-units: unit u -> oc=u%2, t=u//2

# tuning knobs
ACT_EXITS = 63                # body [128,1024] exits via ScalarE (rest: DVE)
TAILU = 4                     # per-oc trailing units grouped oc0-then-oc1
TAIL_A_UNITS = 2              # last units per oc -> Act relu + DVE TT folds
GRP = 4                       # Act slots (1024 wide) per cce-add chain DMA
RING = 16                     # slab-slot ring depth per oc

_compiled = {}


def _unit_order():
    return ([(t, oc) for t in range(NT - TAILU) for oc in (0, 1)]
            + [(t, 0) for t in range(NT - TAILU, NT)]
            + [(t, 1) for t in range(NT - TAILU, NT)])


def _exit_modes():
    """Per-exit 'A' (ScalarE relu -> slab slot) / 'D' (DVE fused relu+acc).
    Bresenham interleave keeps runs short so neither engine bursts; the last
    units of each oc go to DVE so no DMA chain dangles at the end."""
    order = _unit_order()
    ne = 2 * NU
    tail_units = set()
    for oc in (0, 1):
        idxs = [i for i, (t, o) in enumerate(order) if o == oc]
        tail_units.update(idxs[-TAIL_A_UNITS:])
    modes = ["D"] * ne
    body = [e for e in range(ne) if (e // 2) not in tail_units]
    n_act = min(ACT_EXITS, len(body))
    acc = 0
    for e in body:
        acc += n_act
        if acc >= len(body):
            acc -= len(body)
            modes[e] = "A"
    for u in tail_units:
        modes[2 * u] = modes[2 * u + 1] = "T"
    return modes


def _build(chunk_status, repeat=1):
    import concourse.bacc as bacc
    import concourse.mybir as mybir
    import concourse.tile as tile

    f32, f16 = mybir.dt.float32, mybir.dt.float16
    AL = mybir.AluOpType
    AF = mybir.ActivationFunctionType

    nc = bacc.Bacc("TRN2", target_bir_lowering=False, debug=False,
                   num_devices=N_CORES)

    # ---- DRAM I/O (per core) ----
    xT = nc.dram_tensor("xT", [SIN, B], f32, kind="ExternalInput")
    xsatT = nc.dram_tensor("xsatT", [K, B], f32, kind="ExternalInput")
    maskT = nc.dram_tensor("maskT", [SIN, 1], f32, kind="ExternalInput")
    wT = nc.dram_tensor("wT", [SIN, OC], f32, kind="ExternalInput")
    bias2d = nc.dram_tensor("bias2d", [128, 2], f32, kind="ExternalInput")
    w1d = nc.dram_tensor("w1d", [K, 9], f32, kind="ExternalInput")
    b1d = nc.dram_tensor("b1d", [K, 3], f32, kind="ExternalInput")
    w2d = nc.dram_tensor("w2d", [K, 9], f32, kind="ExternalInput")
    b2d = nc.dram_tensor("b2d", [K, 3], f32, kind="ExternalInput")
    w3bd = nc.dram_tensor("w3bd", [128, NT * OC], f32, kind="ExternalInput")
    outT = nc.dram_tensor("outT", [OC, B], f32, kind="ExternalOutput")

    modes = _exit_modes()

    with tile.TileContext(nc) as tc:
      for _rep in range(repeat):
        with (
            tc.tile_pool(name="const", bufs=1) as cp,
            tc.tile_pool(name="stage", bufs=3) as stp,
            tc.tile_pool(name="psumz", bufs=4, space="PSUM") as pp,
        ):
            # ---------- input loads ----------
            # HWDGE: small fp32 tensors
            w1s = cp.tile([K, 9], f32)
            nc.sync.dma_start(w1s[:], w1d.ap())
            v = cp.tile([K, B], f32)
            nc.sync.dma_start(v[:], xsatT.ap())
            b1s = cp.tile([K, 3], f32)
            nc.sync.dma_start(b1s[:], b1d.ap())
            w2s = cp.tile([K, 9], f32)
            nc.sync.dma_start(w2s[:], w2d.ap())
            b2s = cp.tile([K, 3], f32)
            nc.sync.dma_start(b2s[:], b2d.ap())
            bias2 = cp.tile([128, 2], f32)
            nc.sync.dma_start(bias2[:], bias2d.ap())
            masks = cp.tile([128, NCHUNK], f32)
            nc.sync.dma_start(masks[:], maskT.ap().rearrange("(c p) one -> p (c one)", p=128))

            # SWDGE cast loads (fp32 DRAM -> fp16 SBUF)
            x16a = cp.tile([128, 512 * NCHUNK], f16)
            w16a = cp.tile([128, OC * NCHUNK], f16)
            w3b = cp.tile([128, NT * OC], f16)
            QC = NCHUNK // 4

            def load_xw_quarter(q):
                nc.gpsimd.dma_start(
                    x16a[:, 512 * QC * q : 512 * QC * (q + 1)]
                        .rearrange("p (c b) -> p c b", b=B),
                    xT.ap()[128 * QC * q : 128 * QC * (q + 1), :]
                        .rearrange("(c p) b -> p c b", p=128))
                nc.gpsimd.dma_start(
                    w16a[:, OC * QC * q : OC * QC * (q + 1)]
                        .rearrange("p (c o) -> p c o", o=OC),
                    wT.ap()[128 * QC * q : 128 * QC * (q + 1), :]
                        .rearrange("(c p) o -> p c o", p=128))

            load_xw_quarter(0)
            load_xw_quarter(1)
            nc.gpsimd.dma_start(w3b[:], w3bd.ap())

            # prefetch the ACT relu table while DMAs run
            warm = cp.tile([1, 1], f32)
            nc.scalar.activation(warm[:], v[0:1, 0:1], AF.Relu)

            # c1[k, j] = (W1[k,0,j] + W1[k,1,j] + W1[k,2,j]) / 3
            c1a = cp.tile([K, 3], f32)
            nc.vector.tensor_tensor(c1a[:], w1s[:, 0:3], w1s[:, 3:6], AL.add)
            c1 = cp.tile([K, 3], f32)
            nc.vector.tensor_tensor(c1[:], c1a[:], w1s[:, 6:9], AL.add)
            c1s = cp.tile([K, 3], f32)
            nc.vector.tensor_scalar_mul(c1s[:], c1[:], 1.0 / 3.0)
            ones128 = cp.tile([128, 512], f16)
            nc.vector.memset(ones128[:], 1.0)

            # ---------- h path: v -> h1 -> h2 (layout [k, b]) ----------
            h1 = [cp.tile([K, B], f16, tag=f"h1_{j}", name=f"h1_{j}") for j in range(3)]
            for j in range(3):
                nc.scalar.activation(h1[j][:], v[:], AF.Relu,
                                     bias=b1s[:, j:j + 1], scale=c1s[:, j:j + 1])
            h2 = [cp.tile([K, B], f16, tag=f"h2_{j}", name=f"h2_{j}") for j in range(3)]
            for j in range(3):
                ma = stp.tile([K, B], f16, tag="hm0", name=f"hma{j}")
                nc.vector.tensor_scalar_mul(ma[:], h1[0][:], w2s[:, j : j + 1])
                mb = stp.tile([K, B], f16, tag="hm1", name=f"hmb{j}")
                nc.vector.tensor_scalar(mb[:], h1[1][:], w2s[:, 3 + j : 4 + j],
                                        b2s[:, j : j + 1], AL.mult, AL.add)
                sab = stp.tile([K, B], f16, tag="hm2", name=f"hsab{j}")
                nc.vector.tensor_tensor(sab[:], ma[:], mb[:], AL.add)
                mc = stp.tile([K, B], f16, tag="hm1", name=f"hmc{j}")
                nc.vector.tensor_scalar_mul(mc[:], h1[2][:], w2s[:, 6 + j : 7 + j])
                s = stp.tile([K, B], f16, tag="hm0", name=f"hs{j}")
                nc.vector.tensor_tensor(s[:], sab[:], mc[:], AL.add)
                nc.vector.tensor_scalar_max(h2[j][:], s[:], 0.0)

            # ---------- h2t pack: row 32g+j <- h2[j] (buds 32g+t), 4 DMAs ----
            h2t = cp.tile([128, 512 * NT], f16)       # [128, 16384]
            nc.gpsimd.dma_start(
                h2t[3::32, :].rearrange("p (t b) -> p t b", b=512),
                ones128[:],
            )
            for j in range(3):
                nc.gpsimd.dma_start(
                    h2t[j::32, :].rearrange("p (t b) -> p t b", b=512),
                    h2[j][:],
                )
            load_xw_quarter(2)
            load_xw_quarter(3)

            # ---------- dense: bursts through ring tiles, accumulated on DVE -
            live = [c for c in range(NCHUNK) if chunk_status[c] != "full"]
            dsb = cp.tile([128, 1024], f32)
            dgroups = [[c for c in live if c < 8], [c for c in live if c >= 8]]
            dgroups = [g for g in dgroups if g]
            dsb2 = cp.tile([128, 1024], f32)

            def emit_dense(gi):
                burst = dgroups[gi]
                dps = pp.tile([128, 1024], f32, tag="z", name=f"dense{gi}")
                for ci, c in enumerate(burst):
                    x16 = x16a[:, 512 * c : 512 * (c + 1)]
                    if chunk_status[c] == "partial":
                        xm = stp.tile([128, B], f16, tag="x16m", name=f"x16m_{c}_{_rep}")
                        nc.vector.tensor_scalar_mul(xm[:], x16, masks[:, c : c + 1])
                        x16 = xm[:]
                    for oc in range(2):
                        nc.tensor.matmul(dps[:, 512 * oc : 512 * (oc + 1)],
                                         w16a[:, OC * c + 128 * oc : OC * c + 128 * oc + 128],
                                         x16,
                                         start=ci == 0, stop=ci == len(burst) - 1)
                # PSUM -> SBUF on ScalarE (idle at startup); bias on burst 0
                tgt = dsb if gi == 0 else dsb2
                for oc in range(2):
                    nc.scalar.activation(tgt[:, 512 * oc : 512 * (oc + 1)],
                                         dps[:, 512 * oc : 512 * (oc + 1)],
                                         AF.Identity,
                                         bias=bias2[:, oc : oc + 1] if gi == 0 else 0.0)
                if gi == len(dgroups) - 1 and len(dgroups) > 1:
                    nc.gpsimd.tensor_tensor(dsb[:], dsb[:], dsb2[:], AL.add)

            # ---------- bud units ----------
            acc32 = cp.tile([128, 4096], f32)         # 2 oc x 2 lanes x 1024
            acc16 = cp.tile([128, 4096], f16)         # 2 oc x 2 lanes x 1024
            rbslab = cp.tile([128, 2 * RING * 1024], f16)   # 2 oc x RING slots
            n_act = [0, 0]
            first_d = [True, True, True, True]
            first_chain = [True, True]
            chained = [0, 0]
            odd_slots = {0: [], 1: []}

            act_total = [0, 0]
            for i, (t, oc_) in enumerate(_unit_order()):
                for half in range(2):
                    if modes[2 * i + half] == "A":
                        act_total[oc_] += 1
            tailslab = cp.tile([128, 8 * 1024], f16)
            n_tail = [0]

            def emit_chain(oc, base, nslots):
                # 2-slot cce-add sub-DMAs folding slab slots into acc16 lanes
                for s in range(0, nslots - 1, 2):
                    off = (oc * RING + base + s) * 1024
                    rb = rbslab[:, off : off + 2048]
                    aoc = acc16[:, 2048 * oc : 2048 * (oc + 1)]
                    if first_chain[oc]:
                        nc.gpsimd.dma_start(aoc, rb)
                        first_chain[oc] = False
                    else:
                        nc.gpsimd.dma_start(aoc, rb, accum_op=AL.add)
                if nslots % 2:
                    odd_slots[oc].append(base + nslots - 1)

            outsb = cp.tile([128, 1024], f32)

            def emit_tail(oc):
                # trailing slots fold via DVE TTs (no DMA latency), then lane
                # folds + final combine on DVE, then the output store.
                nleft = n_act[oc] - chained[oc]
                a16 = acc16[:, 2048 * oc : 2048 * (oc + 1)]
                for s in range(nleft):
                    slot = (chained[oc] + s) % RING
                    off = (oc * RING + slot) * 1024
                    nc.vector.tensor_tensor(a16[:, 0:1024], a16[:, 0:1024],
                                            rbslab[:, off : off + 1024], AL.add)
                for slot in odd_slots[oc]:
                    off = (oc * RING + slot) * 1024
                    nc.vector.tensor_tensor(a16[:, 0:1024], a16[:, 0:1024],
                                            rbslab[:, off : off + 1024], AL.add)
                nc.vector.tensor_tensor(a16[:, 0:1024], a16[:, 0:1024],
                                        a16[:, 1024:2048], AL.add)
                nc.vector.tensor_tensor(a16[:, 0:512], a16[:, 0:512],
                                        a16[:, 512:1024], AL.add)
                a32 = acc32[:, 2048 * oc : 2048 * (oc + 1)]
                nc.vector.tensor_tensor(a32[:, 0:1024], a32[:, 0:1024],
                                        a32[:, 1024:2048], AL.add)
                nc.vector.tensor_tensor(a32[:, 0:512], a32[:, 0:512],
                                        a32[:, 512:1024], AL.add)
                nc.vector.tensor_tensor(a32[:, 0:512], a32[:, 0:512],
                                        a16[:, 0:512], AL.add)
                ot = outsb[:, 512 * oc : 512 * (oc + 1)]
                nc.vector.tensor_tensor(ot, dsb[:, 512 * oc : 512 * (oc + 1)],
                                        a32[:, 0:512], AL.add)
                nc.sync.dma_start(outT.ap()[128 * oc : 128 * (oc + 1), :], ot)

            unit_order = _unit_order()
            last_unit = {0: max(i for i, (t, o) in enumerate(unit_order) if o == 0),
                         1: len(unit_order) - 1}

            emit_dense(0)
            dense_at = {6: 1}
            for i, (t, oc) in enumerate(unit_order):
                gi = dense_at.get(i)
                if gi is not None and gi < len(dgroups):
                    emit_dense(gi)
                for half in range(2):
                    e = 2 * i + half
                    zt = pp.tile([128, 1024], f32, tag="z", name=f"z{i}_{half}")
                    for gi in range(2):
                        g = 2 * half + gi
                        nc.tensor.matmul(
                            zt[:, 512 * gi : 512 * (gi + 1)],
                            w3b[32 * g : 32 * g + 4,
                                OC * t + 128 * oc : OC * t + 128 * oc + 128],
                            h2t[32 * g : 32 * g + 4, 512 * t : 512 * (t + 1)],
                            start=True, stop=True, tile_position=(32 * g, 0),
                        )
                    if modes[e] == "A":
                        slot = n_act[oc] % RING
                        dst = rbslab[:, (oc * RING + slot) * 1024 :
                                        (oc * RING + slot + 1) * 1024]
                        nc.scalar.activation(dst, zt[:], AF.Relu)
                        n_act[oc] += 1
                        if (n_act[oc] % GRP == 0
                                and n_act[oc] != act_total[oc]):
                            emit_chain(oc, (n_act[oc] - GRP) % RING, GRP)
                            chained[oc] = n_act[oc]
                    elif modes[e] == "T":
                        ts_off = n_tail[0] * 1024
                        n_tail[0] += 1
                        dst = tailslab[:, ts_off : ts_off + 1024]
                        nc.scalar.activation(dst, zt[:], AF.Relu)
                        a16 = acc16[:, 2048 * oc : 2048 * (oc + 1)]
                        nc.vector.tensor_tensor(a16[:, 0:1024], a16[:, 0:1024],
                                                dst, AL.add)
                    else:
                        lane = acc32[:, 2048 * oc + 1024 * half :
                                        2048 * oc + 1024 * (half + 1)]
                        if first_d[oc * 2 + half]:
                            nc.vector.tensor_scalar_max(lane, zt[:], 0.0)
                            first_d[oc * 2 + half] = False
                        else:
                            nc.vector.scalar_tensor_tensor(
                                lane, zt[:], 0.0, lane, op0=AL.max, op1=AL.add)
            emit_tail(0)
            emit_tail(1)
    nc.finalize()
    return nc


def _prep_inputs(x, sat_idx, weight, bias, W1, b1, W2, b2, W3, b3):
    """Host-side shard/layout prep (slicing/permutation only)."""
    x = np.ascontiguousarray(np.asarray(x, np.float32))
    sat = np.asarray(sat_idx).astype(np.int64)
    weight = np.asarray(weight, np.float32)
    bias = np.asarray(bias, np.float32)

    mask = np.ones(SIN, np.float32)
    mask[sat] = 0.0
    chunk_status = []
    for c in range(NCHUNK):
        mc = mask[128 * c : 128 * (c + 1)]
        if not mc.any():
            chunk_status.append("full")
        elif mc.all():
            chunk_status.append("clean")
        else:
            chunk_status.append("partial")
    chunk_status = tuple(chunk_status)

    xT = np.ascontiguousarray(x.T)                       # [SIN, B]
    xsatT = np.ascontiguousarray(x[:, sat].T)            # [K, B]
    maskT = np.ascontiguousarray(mask[:, None])          # [SIN, 1]
    w1h = np.ascontiguousarray(np.asarray(W1, np.float32).reshape(K, 9))
    w2h = np.ascontiguousarray(np.asarray(W2, np.float32).reshape(K, 9))
    b1h = np.ascontiguousarray(np.asarray(b1, np.float32))
    b2h = np.ascontiguousarray(np.asarray(b2, np.float32))
    W3 = np.asarray(W3, np.float32)
    b3 = np.asarray(b3, np.float32)

    in_maps = []
    for cidx in range(N_CORES):
        sl = slice(OC * cidx, OC * (cidx + 1))
        # w3b rows 32g+j hold W3[32g+t, j, sl] over supertiles t; row 32g+3 = b3
        w3bh = np.empty((128, NT, OC), np.float32)
        W3p = W3[:, :, sl]                                # [K, 3, OC]
        b3p = b3[:, sl]                                   # [K, OC]
        for g in range(4):
            buds = np.arange(NT) + 32 * g                 # k = 32g + t
            for j in range(3):
                w3bh[32 * g + j] = W3p[buds, j, :]
            w3bh[32 * g + 3] = b3p[buds, :]
        bias2 = np.ascontiguousarray(bias[sl].reshape(2, 128).T)  # [128, 2]
        in_maps.append({
            "xT": xT,
            "xsatT": xsatT,
            "maskT": maskT,
            "wT": np.ascontiguousarray(weight[sl, :].T),          # [SIN, OC]
            "bias2d": bias2,
            "w1d": w1h, "b1d": b1h, "w2d": w2h, "b2d": b2h,
            "w3bd": np.ascontiguousarray(w3bh.reshape(128, NT * OC)),
        })
    return chunk_status, in_maps


def kernel(**inputs) -> np.ndarray:
    from concourse.bass_utils import run_bass_kernel_spmd

    chunk_status, in_maps = _prep_inputs(
        inputs["x"], inputs["sat_idx"], inputs["weight"], inputs["bias"],
        inputs["W1"], inputs["b1"], inputs["W2"], inputs["b2"],
        inputs["W3"], inputs["b3"],
    )
    if chunk_status not in _compiled:
        _compiled[chunk_status] = _build(chunk_status)
    nc = _compiled[chunk_status]
    res = run_bass_kernel_spmd(nc, in_maps, core_ids=list(range(N_CORES)))
    outT = np.concatenate([res.results[c]["outT"] for c in range(N_CORES)], axis=0)
    return np.ascontiguousarray(outT.T).astype(np.float32)


# revision 80
# speedup vs baseline: 1.0117x; 1.0117x over previous
"""Trainium2 Bass kernel for nn_BuddingLayer (moe_routing).

Computation (B=512, SIN=SOUT=2048, K=128 buds):
  dense = (x * ~mask) @ weight.T + bias          mask = one-hot(sat_idx)
  per bud k (v = x[:, sat_idx[k]]):
    h1 = relu(v * c1[k] + b1[k])                 c1[k,j] = sum_i W1[k,i,j]/3
    h2 = relu(h1 @ W2[k] + b2[k])                [B, 3]
    u += relu(h2 @ W3[k] + b3[k])                [B, 2048]
  out = dense + u

Sharding: output-feature split, 256 columns per core (8 cores); compute in
transposed layout [o_part, b_free].  Host does slicing/permutation only; all
math runs on device.

Bud supertile t packs buds {t, 32+t, 64+t, 96+t} at PE row-group bases
{0,32,64,96}; each bud's 4-row lhsT carries 3 W3 rows + b3 (against a
constant-1 rhs row), so bias rides the matmul for free.  w3b is host-permuted
and loaded by one cast-DMA; h2t needs only 4 pack DMAs (j-row gather).

z flows through a 4-deep [128,1024] PSUM tag-ring (2 matmuls per tile).  The
dense path runs as two paced bursts through the same ring, copied to SBUF by
ScalarE Identity(+bias) in its startup lull and merged once on GpSimd.  Per-exit Bresenham
split of the 128 z-exits:
  'A' exits: ScalarE relu -> fp16 ring slots; slot pairs are summed into
     acc16 lanes by SWDGE cce-add DMAs (sum runs on the DMA engines, off the
     compute engines);
  'D' exits: VectorE fused relu+accumulate (scalar_tensor_tensor max/add)
     into fp32 lanes;
  'T' exits (trailing units, grouped oc0-then-oc1): ScalarE relu + immediate
     VectorE tensor_tensor fold, so no DMA chain dangles at the end.
Lane folds + final dense+bud combine run on VectorE per oc as soon as that
oc's units finish; per-oc output stores overlap the other oc's tail.
"""

import numpy as np

N_CORES = 8
B = 512
SIN = 2048
SOUT = 2048
K = 128
OC = SOUT // N_CORES          # 256 output cols per core
NCHUNK = SIN // 128           # 16 contraction chunks for dense
NT = K // 4                   # 32 supertiles (4 buds each)
NU = 2 * NT                   # 64 z-units: unit u -> oc=u%2, t=u//2

# tuning knobs
ACT_EXITS = 63                # body [128,1024] exits via ScalarE (rest: DVE)
TAILU = 4                     # per-oc trailing units grouped oc0-then-oc1
TAIL_A_UNITS = 2              # last units per oc -> Act relu + DVE TT folds
GRP = 4                       # Act slots (1024 wide) per cce-add chain DMA
RING = 12                     # slab-slot ring depth per oc

_compiled = {}


def _unit_order():
    return ([(t, oc) for t in range(NT - TAILU) for oc in (0, 1)]
            + [(t, 0) for t in range(NT - TAILU, NT)]
            + [(t, 1) for t in range(NT - TAILU, NT)])


def _exit_modes():
    """Per-exit 'A' (ScalarE relu -> slab slot) / 'D' (DVE fused relu+acc).
    Bresenham interleave keeps runs short so neither engine bursts; the last
    units of each oc go to DVE so no DMA chain dangles at the end."""
    order = _unit_order()
    ne = 2 * NU
    tail_units = set()
    for oc in (0, 1):
        idxs = [i for i, (t, o) in enumerate(order) if o == oc]
        tail_units.update(idxs[-TAIL_A_UNITS:])
    modes = ["D"] * ne
    body = [e for e in range(ne) if (e // 2) not in tail_units]
    n_act = min(ACT_EXITS, len(body))
    acc = 0
    for e in body:
        acc += n_act
        if acc >= len(body):
            acc -= len(body)
            modes[e] = "A"
    for u in tail_units:
        modes[2 * u] = modes[2 * u + 1] = "T"
    return modes


def _build(chunk_status, repeat=1):
    import concourse.bacc as bacc
    import concourse.mybir as mybir
    import concourse.tile as tile

    f32, f16 = mybir.dt.float32, mybir.dt.float16
    AL = mybir.AluOpType
    AF = mybir.ActivationFunctionType

    nc = bacc.Bacc("TRN2", target_bir_lowering=False, debug=False,
                   num_devices=N_CORES)

    # ---- DRAM I/O (per core) ----
    xT = nc.dram_tensor("xT", [SIN, B], f32, kind="ExternalInput")
    xsatT = nc.dram_tensor("xsatT", [K, B], f32, kind="ExternalInput")
    maskT = nc.dram_tensor("maskT", [SIN, 1], f32, kind="ExternalInput")
    wT = nc.dram_tensor("wT", [SIN, OC], f32, kind="ExternalInput")
    bias2d = nc.dram_tensor("bias2d", [128, 2], f32, kind="ExternalInput")
    w1d = nc.dram_tensor("w1d", [K, 9], f32, kind="ExternalInput")
    b1d = nc.dram_tensor("b1d", [K, 3], f32, kind="ExternalInput")
    w2d = nc.dram_tensor("w2d", [K, 9], f32, kind="ExternalInput")
    b2d = nc.dram_tensor("b2d", [K, 3], f32, kind="ExternalInput")
    w3bd = nc.dram_tensor("w3bd", [128, NT * OC], f32, kind="ExternalInput")
    outT = nc.dram_tensor("outT", [OC, B], f32, kind="ExternalOutput")

    modes = _exit_modes()

    with tile.TileContext(nc) as tc:
      for _rep in range(repeat):
        with (
            tc.tile_pool(name="const", bufs=1) as cp,
            tc.tile_pool(name="stage", bufs=3) as stp,
            tc.tile_pool(name="psumz", bufs=4, space="PSUM") as pp,
        ):
            # ---------- input loads ----------
            # HWDGE: small fp32 tensors
            w1s = cp.tile([K, 9], f32)
            nc.sync.dma_start(w1s[:], w1d.ap())
            v = cp.tile([K, B], f32)
            nc.sync.dma_start(v[:], xsatT.ap())
            b1s = cp.tile([K, 3], f32)
            nc.sync.dma_start(b1s[:], b1d.ap())
            w2s = cp.tile([K, 9], f32)
            nc.sync.dma_start(w2s[:], w2d.ap())
            b2s = cp.tile([K, 3], f32)
            nc.sync.dma_start(b2s[:], b2d.ap())
            bias2 = cp.tile([128, 2], f32)
            nc.sync.dma_start(bias2[:], bias2d.ap())
            masks = cp.tile([128, NCHUNK], f32)
            nc.sync.dma_start(masks[:], maskT.ap().rearrange("(c p) one -> p (c one)", p=128))

            # SWDGE cast loads (fp32 DRAM -> fp16 SBUF)
            x16a = cp.tile([128, 512 * NCHUNK], f16)
            w16a = cp.tile([128, OC * NCHUNK], f16)
            w3b = cp.tile([128, NT * OC], f16)
            QC = NCHUNK // 4

            def load_xw_quarter(q):
                nc.gpsimd.dma_start(
                    x16a[:, 512 * QC * q : 512 * QC * (q + 1)]
                        .rearrange("p (c b) -> p c b", b=B),
                    xT.ap()[128 * QC * q : 128 * QC * (q + 1), :]
                        .rearrange("(c p) b -> p c b", p=128))
                nc.gpsimd.dma_start(
                    w16a[:, OC * QC * q : OC * QC * (q + 1)]
                        .rearrange("p (c o) -> p c o", o=OC),
                    wT.ap()[128 * QC * q : 128 * QC * (q + 1), :]
                        .rearrange("(c p) o -> p c o", p=128))

            load_xw_quarter(0)
            load_xw_quarter(1)
            nc.gpsimd.dma_start(w3b[:], w3bd.ap())

            # prefetch the ACT relu table while DMAs run
            warm = cp.tile([1, 1], f32)
            nc.scalar.activation(warm[:], v[0:1, 0:1], AF.Relu)

            # c1[k, j] = (W1[k,0,j] + W1[k,1,j] + W1[k,2,j]) / 3
            c1a = cp.tile([K, 3], f32)
            nc.vector.tensor_tensor(c1a[:], w1s[:, 0:3], w1s[:, 3:6], AL.add)
            c1 = cp.tile([K, 3], f32)
            nc.vector.tensor_tensor(c1[:], c1a[:], w1s[:, 6:9], AL.add)
            c1s = cp.tile([K, 3], f32)
            nc.vector.tensor_scalar_mul(c1s[:], c1[:], 1.0 / 3.0)
            ones128 = cp.tile([128, 512], f16)
            nc.vector.memset(ones128[:], 1.0)

            # ---------- h path: v -> h1 -> h2 (layout [k, b]) ----------
            h1 = [cp.tile([K, B], f16, tag=f"h1_{j}", name=f"h1_{j}") for j in range(3)]
            for j in range(3):
                nc.scalar.activation(h1[j][:], v[:], AF.Relu,
                                     bias=b1s[:, j:j + 1], scale=c1s[:, j:j + 1])
            h2 = [cp.tile([K, B], f16, tag=f"h2_{j}", name=f"h2_{j}") for j in range(3)]
            for j in range(3):
                ma = stp.tile([K, B], f16, tag="hm0", name=f"hma{j}")
                nc.vector.tensor_scalar_mul(ma[:], h1[0][:], w2s[:, j : j + 1])
                mb = stp.tile([K, B], f16, tag="hm1", name=f"hmb{j}")
                nc.vector.tensor_scalar(mb[:], h1[1][:], w2s[:, 3 + j : 4 + j],
                                        b2s[:, j : j + 1], AL.mult, AL.add)
                sab = stp.tile([K, B], f16, tag="hm2", name=f"hsab{j}")
                nc.vector.tensor_tensor(sab[:], ma[:], mb[:], AL.add)
                mc = stp.tile([K, B], f16, tag="hm1", name=f"hmc{j}")
                nc.vector.tensor_scalar_mul(mc[:], h1[2][:], w2s[:, 6 + j : 7 + j])
                s = stp.tile([K, B], f16, tag="hm0", name=f"hs{j}")
                nc.vector.tensor_tensor(s[:], sab[:], mc[:], AL.add)
                nc.vector.tensor_scalar_max(h2[j][:], s[:], 0.0)

            # ---------- h2t pack: row 32g+j <- h2[j] (buds 32g+t), 4 DMAs ----
            h2t = cp.tile([128, 512 * NT], f16)       # [128, 16384]
            nc.gpsimd.dma_start(
                h2t[3::32, :].rearrange("p (t b) -> p t b", b=512),
                ones128[:],
            )
            for j in range(3):
                nc.gpsimd.dma_start(
                    h2t[j::32, :].rearrange("p (t b) -> p t b", b=512),
                    h2[j][:],
                )
            load_xw_quarter(2)
            load_xw_quarter(3)

            # ---------- dense: bursts through ring tiles, accumulated on DVE -
            live = [c for c in range(NCHUNK) if chunk_status[c] != "full"]
            dsb = cp.tile([128, 1024], f32)
            dgroups = [[c for c in live if c < 8], [c for c in live if c >= 8]]
            dgroups = [g for g in dgroups if g]
            dsb2 = cp.tile([128, 1024], f32)

            def emit_dense(gi):
                burst = dgroups[gi]
                dps = pp.tile([128, 1024], f32, tag="z", name=f"dense{gi}")
                for ci, c in enumerate(burst):
                    x16 = x16a[:, 512 * c : 512 * (c + 1)]
                    if chunk_status[c] == "partial":
                        xm = stp.tile([128, B], f16, tag="x16m", name=f"x16m_{c}_{_rep}")
                        nc.vector.tensor_scalar_mul(xm[:], x16, masks[:, c : c + 1])
                        x16 = xm[:]
                    for oc in range(2):
                        nc.tensor.matmul(dps[:, 512 * oc : 512 * (oc + 1)],
                                         w16a[:, OC * c + 128 * oc : OC * c + 128 * oc + 128],
                                         x16,
                                         start=ci == 0, stop=ci == len(burst) - 1)
                # PSUM -> SBUF on ScalarE (idle at startup); bias on burst 0
                tgt = dsb if gi == 0 else dsb2
                for oc in range(2):
                    nc.scalar.activation(tgt[:, 512 * oc : 512 * (oc + 1)],
                                         dps[:, 512 * oc : 512 * (oc + 1)],
                                         AF.Identity,
                                         bias=bias2[:, oc : oc + 1] if gi == 0 else 0.0)
                if gi == len(dgroups) - 1 and len(dgroups) > 1:
                    nc.gpsimd.tensor_tensor(dsb[:], dsb[:], dsb2[:], AL.add)

            # ---------- bud units ----------
            acc32 = cp.tile([128, 4096], f32)         # 2 oc x 2 lanes x 1024
            acc16 = cp.tile([128, 4096], f16)         # 2 oc x 2 lanes x 1024
            rbslab = cp.tile([128, 2 * RING * 1024], f16)   # 2 oc x RING slots
            n_act = [0, 0]
            first_d = [True, True, True, True]
            first_chain = [True, True]
            chained = [0, 0]
            odd_slots = {0: [], 1: []}

            act_total = [0, 0]
            for i, (t, oc_) in enumerate(_unit_order()):
                for half in range(2):
                    if modes[2 * i + half] == "A":
                        act_total[oc_] += 1
            tailslab = cp.tile([128, 8 * 1024], f16)
            tailscr = cp.tile([128, 2048], f16)   # per-oc Pool pair-fold out
            n_tail = [0]
            t_slots = {0: [], 1: []}

            def emit_chain(oc, base, nslots):
                # 2-slot cce-add sub-DMAs folding slab slots into acc16 lanes
                for s in range(0, nslots - 1, 2):
                    off = (oc * RING + base + s) * 1024
                    rb = rbslab[:, off : off + 2048]
                    aoc = acc16[:, 2048 * oc : 2048 * (oc + 1)]
                    if first_chain[oc]:
                        nc.gpsimd.dma_start(aoc, rb)
                        first_chain[oc] = False
                    else:
                        nc.gpsimd.dma_start(aoc, rb, accum_op=AL.add)
                if nslots % 2:
                    odd_slots[oc].append(base + nslots - 1)

            outsb = cp.tile([128, 1024], f32)

            def emit_tail(oc):
                # trailing slots fold via DVE TTs (no DMA latency), then lane
                # folds + final combine on DVE, then the output store.
                nleft = n_act[oc] - chained[oc]
                a16 = acc16[:, 2048 * oc : 2048 * (oc + 1)]
                for s in range(nleft):
                    slot = (chained[oc] + s) % RING
                    off = (oc * RING + slot) * 1024
                    nc.vector.tensor_tensor(a16[:, 0:1024], a16[:, 0:1024],
                                            rbslab[:, off : off + 1024], AL.add)
                for slot in odd_slots[oc]:
                    off = (oc * RING + slot) * 1024
                    nc.vector.tensor_tensor(a16[:, 0:1024], a16[:, 0:1024],
                                            rbslab[:, off : off + 1024], AL.add)
                if len(t_slots[oc]) >= 2:
                    nc.vector.tensor_tensor(a16[:, 0:1024], a16[:, 0:1024],
                                            tailscr[:, 1024 * oc : 1024 * (oc + 1)],
                                            AL.add)
                nc.vector.tensor_tensor(a16[:, 0:1024], a16[:, 0:1024],
                                        a16[:, 1024:2048], AL.add)
                nc.vector.tensor_tensor(a16[:, 0:512], a16[:, 0:512],
                                        a16[:, 512:1024], AL.add)
                a32 = acc32[:, 2048 * oc : 2048 * (oc + 1)]
                nc.vector.tensor_tensor(a32[:, 0:1024], a32[:, 0:1024],
                                        a32[:, 1024:2048], AL.add)
                nc.vector.tensor_tensor(a32[:, 0:512], a32[:, 0:512],
                                        a32[:, 512:1024], AL.add)
                nc.vector.tensor_tensor(a32[:, 0:512], a32[:, 0:512],
                                        a16[:, 0:512], AL.add)
                ot = outsb[:, 512 * oc : 512 * (oc + 1)]
                nc.vector.tensor_tensor(ot, dsb[:, 512 * oc : 512 * (oc + 1)],
                                        a32[:, 0:512], AL.add)
                nc.sync.dma_start(outT.ap()[128 * oc : 128 * (oc + 1), :], ot)

            unit_order = _unit_order()
            last_unit = {0: max(i for i, (t, o) in enumerate(unit_order) if o == 0),
                         1: len(unit_order) - 1}

            emit_dense(0)
            dense_at = {6: 1}
            for i, (t, oc) in enumerate(unit_order):
                gi = dense_at.get(i)
                if gi is not None and gi < len(dgroups):
                    emit_dense(gi)
                for half in range(2):
                    e = 2 * i + half
                    zt = pp.tile([128, 1024], f32, tag="z", name=f"z{i}_{half}")
                    for gi in range(2):
                        g = 2 * half + gi
                        nc.tensor.matmul(
                            zt[:, 512 * gi : 512 * (gi + 1)],
                            w3b[32 * g : 32 * g + 4,
                                OC * t + 128 * oc : OC * t + 128 * oc + 128],
                            h2t[32 * g : 32 * g + 4, 512 * t : 512 * (t + 1)],
                            start=True, stop=True, tile_position=(32 * g, 0),
                        )
                    if modes[e] == "A":
                        slot = n_act[oc] % RING
                        dst = rbslab[:, (oc * RING + slot) * 1024 :
                                        (oc * RING + slot + 1) * 1024]
                        nc.scalar.activation(dst, zt[:], AF.Relu)
                        n_act[oc] += 1
                        if (n_act[oc] % GRP == 0
                                and n_act[oc] != act_total[oc]):
                            emit_chain(oc, (n_act[oc] - GRP) % RING, GRP)
                            chained[oc] = n_act[oc]
                    elif modes[e] == "T":
                        ts_off = n_tail[0] * 1024
                        n_tail[0] += 1
                        dst = tailslab[:, ts_off : ts_off + 1024]
                        nc.scalar.activation(dst, zt[:], AF.Relu)
                        t_slots[oc].append(dst)
                        if len(t_slots[oc]) == 2:
                            # first pair folds on GpSimd into a scratch tile
                            # (separate output: Pool must not alias in-place)
                            nc.gpsimd.tensor_tensor(
                                tailscr[:, 1024 * oc : 1024 * (oc + 1)],
                                t_slots[oc][0], t_slots[oc][1], AL.add)
                        elif len(t_slots[oc]) > 2:
                            a16 = acc16[:, 2048 * oc : 2048 * (oc + 1)]
                            nc.vector.tensor_tensor(a16[:, 0:1024],
                                                    a16[:, 0:1024], dst, AL.add)
                    else:
                        lane = acc32[:, 2048 * oc + 1024 * half :
                                        2048 * oc + 1024 * (half + 1)]
                        if first_d[oc * 2 + half]:
                            nc.vector.tensor_scalar_max(lane, zt[:], 0.0)
                            first_d[oc * 2 + half] = False
                        else:
                            nc.vector.scalar_tensor_tensor(
                                lane, zt[:], 0.0, lane, op0=AL.max, op1=AL.add)
            emit_tail(0)
            emit_tail(1)
    nc.finalize()
    return nc


def _prep_inputs(x, sat_idx, weight, bias, W1, b1, W2, b2, W3, b3):
    """Host-side shard/layout prep (slicing/permutation only)."""
    x = np.ascontiguousarray(np.asarray(x, np.float32))
    sat = np.asarray(sat_idx).astype(np.int64)
    weight = np.asarray(weight, np.float32)
    bias = np.asarray(bias, np.float32)

    mask = np.ones(SIN, np.float32)
    mask[sat] = 0.0
    chunk_status = []
    for c in range(NCHUNK):
        mc = mask[128 * c : 128 * (c + 1)]
        if not mc.any():
            chunk_status.append("full")
        elif mc.all():
            chunk_status.append("clean")
        else:
            chunk_status.append("partial")
    chunk_status = tuple(chunk_status)

    xT = np.ascontiguousarray(x.T)                       # [SIN, B]
    xsatT = np.ascontiguousarray(x[:, sat].T)            # [K, B]
    maskT = np.ascontiguousarray(mask[:, None])          # [SIN, 1]
    w1h = np.ascontiguousarray(np.asarray(W1, np.float32).reshape(K, 9))
    w2h = np.ascontiguousarray(np.asarray(W2, np.float32).reshape(K, 9))
    b1h = np.ascontiguousarray(np.asarray(b1, np.float32))
    b2h = np.ascontiguousarray(np.asarray(b2, np.float32))
    W3 = np.asarray(W3, np.float32)
    b3 = np.asarray(b3, np.float32)

    in_maps = []
    for cidx in range(N_CORES):
        sl = slice(OC * cidx, OC * (cidx + 1))
        # w3b rows 32g+j hold W3[32g+t, j, sl] over supertiles t; row 32g+3 = b3
        w3bh = np.empty((128, NT, OC), np.float32)
        W3p = W3[:, :, sl]                                # [K, 3, OC]
        b3p = b3[:, sl]                                   # [K, OC]
        for g in range(4):
            buds = np.arange(NT) + 32 * g                 # k = 32g + t
            for j in range(3):
                w3bh[32 * g + j] = W3p[buds, j, :]
            w3bh[32 * g + 3] = b3p[buds, :]
        bias2 = np.ascontiguousarray(bias[sl].reshape(2, 128).T)  # [128, 2]
        in_maps.append({
            "xT": xT,
            "xsatT": xsatT,
            "maskT": maskT,
            "wT": np.ascontiguousarray(weight[sl, :].T),          # [SIN, OC]
            "bias2d": bias2,
            "w1d": w1h, "b1d": b1h, "w2d": w2h, "b2d": b2h,
            "w3bd": np.ascontiguousarray(w3bh.reshape(128, NT * OC)),
        })
    return chunk_status, in_maps


def kernel(**inputs) -> np.ndarray:
    from concourse.bass_utils import run_bass_kernel_spmd

    chunk_status, in_maps = _prep_inputs(
        inputs["x"], inputs["sat_idx"], inputs["weight"], inputs["bias"],
        inputs["W1"], inputs["b1"], inputs["W2"], inputs["b2"],
        inputs["W3"], inputs["b3"],
    )
    if chunk_status not in _compiled:
        _compiled[chunk_status] = _build(chunk_status)
    nc = _compiled[chunk_status]
    res = run_bass_kernel_spmd(nc, in_maps, core_ids=list(range(N_CORES)))
    outT = np.concatenate([res.results[c]["outT"] for c in range(N_CORES)], axis=0)
    return np.ascontiguousarray(outT.T).astype(np.float32)
